# revision 8
# baseline (speedup 1.0000x reference)
"""DIoU loss (mean) on 8 Trainium2 NeuronCores via Bass/Tile.

Sharding: boxes [2e6, 4] are interleaved host-side into one per-core
tensor X [128, W, 8] = (pred xyxy || targ xyxy) per box (partition-major;
the tail padded with identity boxes whose contribution is subtracted on
the host). Each core reduces its slice to per-chunk partial sums of
iou and cd/diag; the host finishes the mean in float64.

Math (per axis a, sums/diffs basis; p1,p2,t1,t2 box edges):
  S_p = p1+p2, D_p = p2-p1, S_t = t1+t2, D_t = t2-t1
  d = S_p - S_t           (2x center diff)
  q = D_t - D_p,  g = D_p + D_t
  h = max(|d|, |q|)       (= |p1-t1| + |p2-t2|)
  u2 = g-h (2*overlap), e2 = g+h (2*enclosing extent)
  inter2 = relu(u2_x)*relu(u2_y)/2
  area_p+area_t = (g_x*g_y + q_x*q_y)/2   -> union2 = (gg+qq) - inter2
  cd4 = d_x^2+d_y^2,  diag4 = e2_x^2+e2_y^2
  loss_i = 1 - inter2/union2 + cd4/diag4
Divisions run in log space on ACT (Ln then Exp with accumulate); the
bulk elementwise stream runs in fp16 on DVE (2x mode), targ first-touch
on GPSIMD tensor ops, pred cast + Ln/Exp on ACT (single act-table).
"""

import numpy as np

import concourse.bass as bass
import concourse.mybir as mybir
from concourse import bacc
from concourse.tile import TileContext
from concourse.bass_utils import run_bass_kernel_spmd

N_BOXES = 2_000_000
P = 128
COLS = N_BOXES // P            # 15625
N_CORES = 8
W = 1956                       # columns per core (8*1956 = 15648 >= 15625)
PAD_BOXES = N_CORES * W * P - N_BOXES  # 2944

F32 = mybir.dt.float32
F16 = mybir.dt.float16
ALU = mybir.AluOpType
AF = mybir.ActivationFunctionType

_CACHE = {}


def _register_custom_ops():
    """Register fused DVE ops (idempotent); self-pin uops_sha."""
    import concourse.dve_ops as dve_ops_mod
    from concourse.dve_spec import Spec, Src0, Src1, Zero, C2, maxx, relu, sq, lower
    from concourse.dve_ops import OPS, DveOp, has_src1
    from concourse.dve_uop import DveOpSpec

    def reg(name, spec):
        for op in OPS:
            if op.name == name:
                return op
        op = DveOp(name, spec, subdim=False, uops_sha={})
        OPS.append(op)
        row = dve_ops_mod._CUSTOM_DVE_ROW_BASE + len(OPS) - 1
        assert row < 0x20, "custom-DVE row field overflow"
        dve_ops_mod._SUB_OPCODE_FOR_NAME[name] = row
        dve_ops_mod.CUSTOM_DVE_SPECS[name] = spec
        for ver in ("v3", "v4"):
            sp = DveOpSpec(name=name, opcode=row, uops=lower(spec, ver=ver),
                           rd1_en=has_src1(spec))
            op.uops_sha[ver] = sp.sha(ver)
        return op

    absmax2 = reg("ANT_ABSMAX2", Spec(
        body=maxx(maxx(Src0, Zero - Src0), maxx(Src1, Zero - Src1)),
        reference=lambda in0, in1: np.maximum(np.abs(in0), np.abs(in1))))
    relumul2 = reg("ANT_RELUMUL2", Spec(
        body=relu(Src0) * relu(Src1) * C2,
        reference=lambda in0, in1, imm2=0.0:
            np.maximum(in0, 0) * np.maximum(in1, 0) * imm2))
    sq2sum = reg("ANT_SQ2SUM", Spec(
        body=sq(Src0) + sq(Src1),
        reference=lambda in0, in1: in0 * in0 + in1 * in1))
    return absmax2, relumul2, sq2sum


def _build_program(chunks=None, bio=2, bwk=2, lag=1,
                   u2_split=0.0, d_split=0.0, delta_split=0.0,
                   union_pool=True, ps_pool=False, gacc=False, e2acc=False):
    """*_split: fraction of that op's columns on Pool (rest DVE).
    union_pool/ps_pool: engine choice. gacc/e2acc: DMA-accum adds."""
    if chunks is None:
        chunks = [96, 240, 400, 400, 410, 410]
    assert sum(chunks) == W
    nch = len(chunks)
    offs = [sum(chunks[:i]) for i in range(nch)]
    nc = bacc.Bacc(None, target_bir_lowering=False)

    x_d = nc.dram_tensor("x", [P, W, 8], F32, kind="ExternalInput")
    acc_d = nc.dram_tensor("acc", [P, 2 * nch], F32, kind="ExternalOutput")

    dve = nc.vector
    gp = nc.gpsimd
    act = nc.scalar
    ABSMAX2, RELUMUL2, SQ2SUM = _register_custom_ops()

    with TileContext(nc) as tc:
        with (
            tc.tile_pool(name="io", bufs=bio) as io,
            tc.tile_pool(name="wk", bufs=bwk) as wk,
            tc.tile_pool(name="accp", bufs=1) as accp,
        ):
            acc = accp.tile([P, 2 * nch], F32)
            lnb = accp.tile([P, 1], F32)
            gp.memset(lnb[:], 1e-30)
            # table 6 (natural_log_exp_and_others) holds copy+ln+exp, so one
            # up-front load defeats the greedy per-func table thrash
            act.add_instruction(mybir.InstLoadActFuncSet(
                name=nc.get_next_instruction_name(),
                act_func_set_id=6, ins=[], outs=[]))
            state = {}

            def split2(fc, frac, pool_op, dve_op):
                sp = int(fc * frac)
                if sp > 0:
                    pool_op(sp)
                if sp < fc:
                    dve_op(sp)

            def front(i):
                fc = chunks[i]
                o0 = offs[i]
                x = io.tile([P, fc, 8], F32, tag="x")
                nc.sync.dma_start(out=x[:], in_=x_d[:, o0:o0 + fc, :])

                # targ first-touch on GPSIMD (plain tensor ops, f32 -> fp16)
                st = wk.tile([P, fc, 2], F16, tag="st")
                gp.tensor_add(st[:], x[:, :, 4:6], x[:, :, 6:8])         # S_t
                dt = wk.tile([P, fc, 2], F16, tag="dt")
                gp.tensor_sub(dt[:], x[:, :, 6:8], x[:, :, 4:6])         # D_t

                # pred cast on ACT, then 2x fp16 first-touch on DVE
                xp = wk.tile([P, fc, 4], F16, tag="xp")
                act.activation(xp[:], x[:, :, 0:4], AF.Copy)
                sp_ = wk.tile([P, fc, 2], F16, tag="sp")
                dve.tensor_add(sp_[:], xp[:, :, 0:2], xp[:, :, 2:4])     # S_p
                gt = wk.tile([P, fc, 2], F16, tag="gt")
                dve.tensor_sub(gt[:], xp[:, :, 2:4], xp[:, :, 0:2])      # D_p

                # d = S_p - S_t
                dd = wk.tile([P, fc, 2], F16, tag="dd")
                split2(fc, d_split,
                       lambda s: gp.tensor_sub(dd[:, :s], sp_[:, :s], st[:, :s]),
                       lambda s: dve.tensor_sub(dd[:, s:], sp_[:, s:], st[:, s:]))
                # q = D_t - D_p   (gt holds D_p until the g accumulate)
                qt = wk.tile([P, fc, 2], F16, tag="qt")
                dve.tensor_sub(qt[:], dt[:], gt[:])
                # g = D_p + D_t
                if gacc:
                    gp.dma_start(out=gt[:], in_=dt[:], accum_op=ALU.add)
                else:
                    dve.tensor_add(gt[:], gt[:], dt[:])

                ht = wk.tile([P, fc, 2], F16, tag="ht")
                dve._custom_dve(ABSMAX2, out=ht[:], in0=dd[:], in1=qt[:])
                u2 = wk.tile([P, fc, 2], F16, tag="u2")
                split2(fc, u2_split,
                       lambda s: gp.tensor_sub(u2[:, :s], gt[:, :s], ht[:, :s]),
                       lambda s: dve.tensor_sub(u2[:, s:], gt[:, s:], ht[:, s:]))

                # f32 quad (inter2, cd4, union2, diag4)
                f4 = wk.tile([P, fc, 4], F32, tag="f4")
                dve._custom_dve(RELUMUL2, out=f4[:, :, 0],
                                in0=u2[:, :, 0], in1=u2[:, :, 1], imm2=0.5)
                dve._custom_dve(SQ2SUM, out=f4[:, :, 1],
                                in0=dd[:, :, 0], in1=dd[:, :, 1])        # cd4

                # e2 = g + h (in place over h)
                if e2acc:
                    gp.dma_start(out=ht[:], in_=gt[:], accum_op=ALU.add)
                else:
                    dve.tensor_add(ht[:], gt[:], ht[:])
                dve._custom_dve(SQ2SUM, out=f4[:, :, 3],
                                in0=ht[:, :, 0], in1=ht[:, :, 1])        # diag4

                # areas: gg+qq = 2*(area_p+area_t)
                pr = wk.tile([P, fc, 2], F16, tag="pr")
                dve.tensor_mul(pr[:, :, 0], gt[:, :, 0], gt[:, :, 1])
                dve.tensor_mul(pr[:, :, 1], qt[:, :, 0], qt[:, :, 1])
                ps = wk.tile([P, fc], F16, tag="ps")
                if ps_pool:
                    gp.tensor_add(ps[:], pr[:, :, 0], pr[:, :, 1])
                else:
                    dve.tensor_add(ps[:], pr[:, :, 0], pr[:, :, 1])
                if union_pool:
                    gp.tensor_sub(f4[:, :, 2], ps[:], f4[:, :, 0])       # union2
                else:
                    dve.scalar_tensor_tensor(
                        out=f4[:, :, 2], in0=ps[:], scalar=1.0,
                        in1=f4[:, :, 0], op0=ALU.mult, op1=ALU.subtract)
                state[i] = f4

            def back(i):
                fc = chunks[i]
                f4 = state.pop(i)
                l4 = wk.tile([P, fc, 4], F16, tag="l4")
                act.activation(l4[:], f4[:], AF.Ln, bias=lnb[:, 0:1])
                delta = wk.tile([P, fc, 2], F16, tag="delta")
                split2(fc, delta_split,
                       lambda s: gp.tensor_sub(delta[:, :s], l4[:, :s, 0:2],
                                               l4[:, :s, 2:4]),
                       lambda s: dve.tensor_sub(delta[:, s:], l4[:, s:, 0:2],
                                                l4[:, s:, 2:4]))
                r = wk.tile([P, fc, 2], F16, tag="r")
                act.activation(r[:, :, 0], delta[:, :, 0], AF.Exp,
                               accum_out=acc[:, 2 * i:2 * i + 1])
                act.activation(r[:, :, 1], delta[:, :, 1], AF.Exp,
                               accum_out=acc[:, 2 * i + 1:2 * i + 2])

            for i in range(nch + lag):
                if i < nch:
                    front(i)
                if i >= lag:
                    back(i - lag)

            nc.sync.dma_start(out=acc_d[:], in_=acc[:])

    nc.finalize()
    return nc


def _shard(pred, targ):
    """pred/targ [N_BOXES,4] -> list of 8 per-core [P, W, 8] interleaved."""
    p = np.ascontiguousarray(pred, dtype=np.float32).reshape(P, COLS, 4)
    t = np.ascontiguousarray(targ, dtype=np.float32).reshape(P, COLS, 4)
    full = np.empty((P, N_CORES * W, 8), dtype=np.float32)
    full[:, :COLS, 0:4] = p
    full[:, :COLS, 4:8] = t
    full[:, COLS:, 0:4] = np.array([0.0, 0.0, 1.0, 1.0], dtype=np.float32)
    full[:, COLS:, 4:8] = np.array([0.0, 0.0, 1.0, 1.0], dtype=np.float32)
    return [np.ascontiguousarray(full[:, c * W:(c + 1) * W, :])
            for c in range(N_CORES)]


def kernel(pred_boxes, target_boxes):
    if "nc" not in _CACHE:
        _CACHE["nc"] = _build_program()
    nc = _CACHE["nc"]

    xs = _shard(np.asarray(pred_boxes), np.asarray(target_boxes))
    in_maps = [{"x": xs[c]} for c in range(N_CORES)]

    # the device occasionally reports a transient NRT_EXEC_UNIT_UNRECOVERABLE
    # wedge; it clears on re-execution, so retry a few times
    last_err = None
    for _attempt in range(4):
        try:
            res = run_bass_kernel_spmd(nc, in_maps, list(range(N_CORES)))
            break
        except Exception as e:
            last_err = e
    else:
        raise last_err

    # acc columns: even = sum(iou), odd = sum(cd/diag) per chunk
    s_iou = 0.0
    s_cdr = 0.0
    for c in range(N_CORES):
        a = res.results[c]["acc"].astype(np.float64)
        s_iou += a[:, 0::2].sum()
        s_cdr += a[:, 1::2].sum()
    # padded identity boxes contribute iou=1, cd/diag=0 each
    s_iou -= float(PAD_BOXES)
    loss = 1.0 - (s_iou - s_cdr) / float(N_BOXES)
    return np.float32(loss)


# revision 9
# speedup vs baseline: 1.4358x; 1.4358x over previous
"""DIoU loss (mean) on 8 Trainium2 NeuronCores via Bass/Tile.

Sharding: boxes [2e6, 4] are interleaved host-side into one per-core
tensor X [128, W, 8] = (pred xyxy || targ xyxy) per box (partition-major;
the tail padded with identity boxes whose contribution is subtracted on
the host). Each core reduces its slice to per-chunk partial sums of
iou and cd/diag; the host finishes the mean in float64.

Math (per axis a, sums/diffs basis; p1,p2,t1,t2 box edges):
  S_p = p1+p2, D_p = p2-p1, S_t = t1+t2, D_t = t2-t1
  d = S_p - S_t           (2x center diff)
  q = D_t - D_p,  g = D_p + D_t
  h = max(|d|, |q|)       (= |p1-t1| + |p2-t2|)
  u2 = g-h (2*overlap), e2 = g+h (2*enclosing extent)
  inter2 = relu(u2_x)*relu(u2_y)/2
  area_p+area_t = (g_x*g_y + q_x*q_y)/2   -> union2 = (gg+qq) - inter2
  cd4 = d_x^2+d_y^2,  diag4 = e2_x^2+e2_y^2
  loss_i = 1 - inter2/union2 + cd4/diag4
Divisions run in log space on ACT (Ln then Exp with accumulate); the
bulk elementwise stream runs in fp16 on DVE (2x mode), targ first-touch
on GPSIMD tensor ops, pred cast + Ln/Exp on ACT (single act-table).
"""

import numpy as np

import concourse.bass as bass
import concourse.mybir as mybir
from concourse import bacc
from concourse.tile import TileContext
from concourse.bass_utils import run_bass_kernel_spmd

N_BOXES = 2_000_000
P = 128
COLS = N_BOXES // P            # 15625
N_CORES = 8
W = 1956                       # columns per core (8*1956 = 15648 >= 15625)
PAD_BOXES = N_CORES * W * P - N_BOXES  # 2944

F32 = mybir.dt.float32
F16 = mybir.dt.float16
ALU = mybir.AluOpType
AF = mybir.ActivationFunctionType

_CACHE = {}


def _register_custom_ops():
    """Register fused DVE ops (idempotent); self-pin uops_sha."""
    import concourse.dve_ops as dve_ops_mod
    from concourse.dve_spec import Spec, Src0, Src1, Zero, C2, maxx, relu, sq, lower
    from concourse.dve_ops import OPS, DveOp, has_src1
    from concourse.dve_uop import DveOpSpec

    def reg(name, spec):
        for op in OPS:
            if op.name == name:
                return op
        op = DveOp(name, spec, subdim=False, uops_sha={})
        OPS.append(op)
        row = dve_ops_mod._CUSTOM_DVE_ROW_BASE + len(OPS) - 1
        assert row < 0x20, "custom-DVE row field overflow"
        dve_ops_mod._SUB_OPCODE_FOR_NAME[name] = row
        dve_ops_mod.CUSTOM_DVE_SPECS[name] = spec
        for ver in ("v3", "v4"):
            sp = DveOpSpec(name=name, opcode=row, uops=lower(spec, ver=ver),
                           rd1_en=has_src1(spec))
            op.uops_sha[ver] = sp.sha(ver)
        return op

    absmax2 = reg("ANT_ABSMAX2", Spec(
        body=maxx(maxx(Src0, Zero - Src0), maxx(Src1, Zero - Src1)),
        reference=lambda in0, in1: np.maximum(np.abs(in0), np.abs(in1))))
    relumul2 = reg("ANT_RELUMUL2", Spec(
        body=relu(Src0) * relu(Src1) * C2,
        reference=lambda in0, in1, imm2=0.0:
            np.maximum(in0, 0) * np.maximum(in1, 0) * imm2))
    sq2sum = reg("ANT_SQ2SUM", Spec(
        body=sq(Src0) + sq(Src1),
        reference=lambda in0, in1: in0 * in0 + in1 * in1))
    return absmax2, relumul2, sq2sum


def _build_program(chunks=None, bio=2, bwk=2, lag=1,
                   u2_split=0.0, d_split=0.0, delta_split=0.0,
                   union_pool=True, ps_pool=False, gacc=False, e2acc=False):
    """*_split: fraction of that op's columns on Pool (rest DVE).
    union_pool/ps_pool: engine choice. gacc/e2acc: DMA-accum adds."""
    if chunks is None:
        chunks = [96, 240, 400, 400, 410, 410]
    assert sum(chunks) == W
    nch = len(chunks)
    offs = [sum(chunks[:i]) for i in range(nch)]
    nc = bacc.Bacc(None, target_bir_lowering=False)

    x_d = nc.dram_tensor("x", [P, W, 8], F32, kind="ExternalInput")
    acc_d = nc.dram_tensor("acc", [P, 2 * nch], F32, kind="ExternalOutput")

    dve = nc.vector
    gp = nc.gpsimd
    act = nc.scalar
    ABSMAX2, RELUMUL2, SQ2SUM = _register_custom_ops()

    with TileContext(nc) as tc:
        with (
            tc.tile_pool(name="io", bufs=bio) as io,
            tc.tile_pool(name="wk", bufs=bwk) as wk,
            tc.tile_pool(name="accp", bufs=1) as accp,
        ):
            acc = accp.tile([P, 2 * nch], F32)
            lnb = accp.tile([P, 1], F32)
            gp.memset(lnb[:], 1e-30)
            # table 6 (natural_log_exp_and_others) holds copy+ln+exp, so one
            # up-front load defeats the greedy per-func table thrash
            act.add_instruction(mybir.InstLoadActFuncSet(
                name=nc.get_next_instruction_name(),
                act_func_set_id=6, ins=[], outs=[]))
            state = {}

            def split2(fc, frac, pool_op, dve_op):
                sp = int(fc * frac)
                if sp > 0:
                    pool_op(sp)
                if sp < fc:
                    dve_op(sp)

            def front(i):
                fc = chunks[i]
                o0 = offs[i]
                x = io.tile([P, fc, 8], F32, tag="x")
                nc.sync.dma_start(out=x[:], in_=x_d[:, o0:o0 + fc, :])

                # targ first-touch on GPSIMD (plain tensor ops, f32 -> fp16)
                st = wk.tile([P, fc, 2], F16, tag="st")
                gp.tensor_add(st[:], x[:, :, 4:6], x[:, :, 6:8])         # S_t
                dt = wk.tile([P, fc, 2], F16, tag="dt")
                gp.tensor_sub(dt[:], x[:, :, 6:8], x[:, :, 4:6])         # D_t

                # pred cast on ACT, then 2x fp16 first-touch on DVE
                xp = wk.tile([P, fc, 4], F16, tag="xp")
                act.activation(xp[:], x[:, :, 0:4], AF.Copy)
                sp_ = wk.tile([P, fc, 2], F16, tag="sp")
                dve.tensor_add(sp_[:], xp[:, :, 0:2], xp[:, :, 2:4])     # S_p
                gt = wk.tile([P, fc, 2], F16, tag="gt")
                dve.tensor_sub(gt[:], xp[:, :, 2:4], xp[:, :, 0:2])      # D_p

                # d = S_p - S_t
                dd = wk.tile([P, fc, 2], F16, tag="dd")
                split2(fc, d_split,
                       lambda s: gp.tensor_sub(dd[:, :s], sp_[:, :s], st[:, :s]),
                       lambda s: dve.tensor_sub(dd[:, s:], sp_[:, s:], st[:, s:]))
                # q = D_t - D_p   (gt holds D_p until the g add), into gq lanes
                dve.tensor_sub(gt[:, :, 2:4], dt[:], gt[:, :, 0:2])
                # g = D_p + D_t (in place over D_p)
                dve.tensor_add(gt[:, :, 0:2], gt[:, :, 0:2], dt[:])

                ht = wk.tile([P, fc, 2], F16, tag="ht")
                dve._custom_dve(ABSMAX2, out=ht[:], in0=dd[:],
                                in1=gt[:, :, 2:4])
                u2 = wk.tile([P, fc, 2], F16, tag="u2")
                split2(fc, u2_split,
                       lambda s: gp.tensor_sub(u2[:, :s], gt[:, :s], ht[:, :s]),
                       lambda s: dve.tensor_sub(u2[:, s:], gt[:, s:], ht[:, s:]))

                # f32 quad (inter2, cd4, union2, diag4)
                f4 = wk.tile([P, fc, 4], F32, tag="f4")
                dve._custom_dve(RELUMUL2, out=f4[:, :, 0],
                                in0=u2[:, :, 0], in1=u2[:, :, 1], imm2=0.5)
                dve._custom_dve(SQ2SUM, out=f4[:, :, 1],
                                in0=dd[:, :, 0], in1=dd[:, :, 1])        # cd4

                # e2 = g + h (in place over h)
                if e2acc:
                    gp.dma_start(out=ht[:], in_=gt[:], accum_op=ALU.add)
                else:
                    dve.tensor_add(ht[:], gt[:], ht[:])
                dve._custom_dve(SQ2SUM, out=f4[:, :, 3],
                                in0=ht[:, :, 0], in1=ht[:, :, 1])        # diag4

                # areas: gg+qq = 2*(area_p+area_t)
                pr = wk.tile([P, fc, 2], F16, tag="pr")
                dve.tensor_mul(pr[:, :, 0], gt[:, :, 0], gt[:, :, 1])
                dve.tensor_mul(pr[:, :, 1], qt[:, :, 0], qt[:, :, 1])
                ps = wk.tile([P, fc], F16, tag="ps")
                if ps_pool:
                    gp.tensor_add(ps[:], pr[:, :, 0], pr[:, :, 1])
                else:
                    dve.tensor_add(ps[:], pr[:, :, 0], pr[:, :, 1])
                if union_pool:
                    gp.tensor_sub(f4[:, :, 2], ps[:], f4[:, :, 0])       # union2
                else:
                    dve.scalar_tensor_tensor(
                        out=f4[:, :, 2], in0=ps[:], scalar=1.0,
                        in1=f4[:, :, 0], op0=ALU.mult, op1=ALU.subtract)
                state[i] = f4

            def back(i):
                fc = chunks[i]
                f4 = state.pop(i)
                l4 = wk.tile([P, fc, 4], F16, tag="l4")
                act.activation(l4[:], f4[:], AF.Ln, bias=lnb[:, 0:1])
                delta = wk.tile([P, fc, 2], F16, tag="delta")
                split2(fc, delta_split,
                       lambda s: gp.tensor_sub(delta[:, :s], l4[:, :s, 0:2],
                                               l4[:, :s, 2:4]),
                       lambda s: dve.tensor_sub(delta[:, s:], l4[:, s:, 0:2],
                                                l4[:, s:, 2:4]))
                r = wk.tile([P, fc, 2], F16, tag="r")
                act.activation(r[:, :, 0], delta[:, :, 0], AF.Exp,
                               accum_out=acc[:, 2 * i:2 * i + 1])
                act.activation(r[:, :, 1], delta[:, :, 1], AF.Exp,
                               accum_out=acc[:, 2 * i + 1:2 * i + 2])

            for i in range(nch + lag):
                if i < nch:
                    front(i)
                if i >= lag:
                    back(i - lag)

            nc.sync.dma_start(out=acc_d[:], in_=acc[:])

    nc.finalize()
    return nc


def _shard(pred, targ):
    """pred/targ [N_BOXES,4] -> list of 8 per-core [P, W, 8] interleaved."""
    p = np.ascontiguousarray(pred, dtype=np.float32).reshape(P, COLS, 4)
    t = np.ascontiguousarray(targ, dtype=np.float32).reshape(P, COLS, 4)
    full = np.empty((P, N_CORES * W, 8), dtype=np.float32)
    full[:, :COLS, 0:4] = p
    full[:, :COLS, 4:8] = t
    full[:, COLS:, 0:4] = np.array([0.0, 0.0, 1.0, 1.0], dtype=np.float32)
    full[:, COLS:, 4:8] = np.array([0.0, 0.0, 1.0, 1.0], dtype=np.float32)
    return [np.ascontiguousarray(full[:, c * W:(c + 1) * W, :])
            for c in range(N_CORES)]


def kernel(pred_boxes, target_boxes):
    if "nc" not in _CACHE:
        _CACHE["nc"] = _build_program()
    nc = _CACHE["nc"]

    xs = _shard(np.asarray(pred_boxes), np.asarray(target_boxes))
    in_maps = [{"x": xs[c]} for c in range(N_CORES)]

    # the device occasionally reports a transient NRT_EXEC_UNIT_UNRECOVERABLE
    # wedge; it clears on re-execution, so retry a few times
    last_err = None
    for _attempt in range(4):
        try:
            res = run_bass_kernel_spmd(nc, in_maps, list(range(N_CORES)))
            break
        except Exception as e:
            last_err = e
    else:
        raise last_err

    # acc columns: even = sum(iou), odd = sum(cd/diag) per chunk
    s_iou = 0.0
    s_cdr = 0.0
    for c in range(N_CORES):
        a = res.results[c]["acc"].astype(np.float64)
        s_iou += a[:, 0::2].sum()
        s_cdr += a[:, 1::2].sum()
    # padded identity boxes contribute iou=1, cd/diag=0 each
    s_iou -= float(PAD_BOXES)
    loss = 1.0 - (s_iou - s_cdr) / float(N_BOXES)
    return np.float32(loss)
